# revision 8
# baseline (speedup 1.0000x reference)
"""Social-LSTM single-step kernel for 8 Trainium2 NeuronCores.

Strategy: shard the 1024 target pedestrians row-wise across 8 cores
(128 targets each); every core sees all 1024 neighbors. The [N,N,64]
one-hot grid mask is never materialized in HBM: each core builds, on
chip, bf16 equality masks against a code ramp and feeds them as the
moving operand of TensorE matmuls with the neighbor hidden states as
the stationary operand, accumulating social^T[h, n] per grid cell in
PSUM. The social pooling, embedding, LSTM cell and output projection
run entirely on-chip per core; the host only slices/permutes inputs
and concatenates the 8 output shards.
"""
import numpy as np
import ml_dtypes

from concourse import bass, mybir
from concourse.tile import TileContext, ScopedClock
from concourse.bass_utils import run_bass_kernel_spmd

F32 = mybir.dt.float32
I32 = mybir.dt.int32
BF16 = mybir.dt.bfloat16
ALU = mybir.AluOpType
ACT = mybir.ActivationFunctionType
BF = ml_dtypes.bfloat16

N = 1024
RNN = 128
EMB = 64
GS = 8
G = GS * GS          # 64 grid cells
NMIX = 20
NCORE = 8
NC_CHUNK = N // NCORE  # 128 targets per core
MC = 8                 # neighbor chunks of 128
G_GRP = 16             # grid cells per PSUM group
N_GRP = G // G_GRP     # 4 groups

# mask/social/pool-weight dtype (bf16: exact 0/1 masks, 2x DVE mode)
MDT = BF16
MNP = BF


# ---------------------------------------------------------------------------
# walrus in this env accepts only ONE sync wait per instruction: patch the
# TileContext tail drain and split any other multi-wait instruction.
def _patched_drain(self, tick_clock, wait_clock):
    nop_inst = self.nc.sync.nop()
    wait_clock.add_sem_waits(nop_inst.ins, ScopedClock({None: tick_clock.global_clock}))
    si = nop_inst.ins.sync_info
    waits = list(si.on_wait or [])
    si.on_wait = waits[:1]
    for i in range(1, len(waits)):
        extra = self.nc.sync.nop()
        extra.ins.sync_info = mybir.SyncInfo(on_update=[], on_wait=[waits[i]])
    self.nc.sync.drain()
    self.nc.all_engine_barrier()
    popped = self.nc._tile_sem_poison_stack.pop()
    assert popped is self._sem_poison
    self.nc.clear_and_free_semaphores(list(self.sems.allocated().values()))
    self.nc.all_engine_barrier()


TileContext._drain_and_barrier = _patched_drain


def _split_multi_waits(nc):
    for fn in nc.m.functions:
        for bb in fn.blocks:
            new_insts = []
            for inst in bb.instructions:
                si = getattr(inst, "sync_info", None)
                waits = list(si.on_wait) if si is not None and si.on_wait else []
                if len(waits) > 1:
                    for w in waits[:-1]:
                        new_insts.append(mybir.InstNoOp(
                            name=nc.get_next_instruction_name(), ins=[], outs=[],
                            engine=inst.engine,
                            sync_info=mybir.SyncInfo(on_update=[], on_wait=[w]),
                        ))
                    si.on_wait = [waits[-1]]
                new_insts.append(inst)
            bb.instructions = new_insts


# ---------------------------------------------------------------------------
def _register_const(nc, dtype, val):
    t = nc.alloc_sbuf_tensor(f"const-{dtype.name}-{val}", [128, 1], dtype)
    nc.gpsimd.memset(t.ap(), val)
    nc.const_aps.aps[(dtype, val)] = t.ap()


def _build_program():
    nc = bass.Bass(target_bir_lowering=False)
    _register_const(nc, F32, 0.5)
    _register_const(nc, F32, 9.0)
    nc.all_engine_barrier()

    # --- per-core external inputs ---
    xabs_r = nc.dram_tensor("xabs_r", [128, 2 * MC], F32, kind="ExternalInput")
    xnb = nc.dram_tensor("xnb", [128, NC_CHUNK], F32, kind="ExternalInput")
    ynb = nc.dram_tensor("ynb", [128, NC_CHUNK], F32, kind="ExternalInput")
    eye_r = nc.dram_tensor("eye_r", [128, N], MDT, kind="ExternalInput")
    h_all = nc.dram_tensor("h_all", [N, RNN], MDT, kind="ExternalInput")
    wsoc_r = nc.dram_tensor("wsoc_r", [RNN, G * EMB], MDT, kind="ExternalInput")
    wembT = nc.dram_tensor("wembT", [2, EMB], F32, kind="ExternalInput")
    xoffT = nc.dram_tensor("xoffT", [2, NC_CHUNK], F32, kind="ExternalInput")
    b_embsoc = nc.dram_tensor("b_embsoc", [128, 1], F32, kind="ExternalInput")
    wihT = nc.dram_tensor("wihT", [128, 4 * RNN], F32, kind="ExternalInput")
    whhT = nc.dram_tensor("whhT", [RNN, 4 * RNN], F32, kind="ExternalInput")
    bgates_ih = nc.dram_tensor("bgates_ih", [128, 4], F32, kind="ExternalInput")
    bgates_hh = nc.dram_tensor("bgates_hh", [128, 4], F32, kind="ExternalInput")
    hT_c = nc.dram_tensor("hT_c", [RNN, NC_CHUNK], F32, kind="ExternalInput")
    cT_c = nc.dram_tensor("cT_c", [RNN, NC_CHUNK], F32, kind="ExternalInput")
    woutT = nc.dram_tensor("woutT", [RNN, 6 * NMIX], F32, kind="ExternalInput")
    bout = nc.dram_tensor("bout", [6 * NMIX, 1], F32, kind="ExternalInput")
    outT = nc.dram_tensor("outT", [6 * NMIX, NC_CHUNK], F32, kind="ExternalOutput")

    with TileContext(nc) as tc:
        with (
            tc.tile_pool(name="const", bufs=1) as cpool,
            tc.tile_pool(name="cells", bufs=1) as cellpool,
            tc.tile_pool(name="masks", bufs=3) as maskpool,
            tc.tile_pool(name="soc", bufs=2) as socpool,
            tc.tile_pool(name="work", bufs=2) as work,
            tc.tile_pool(name="psum", bufs=1, space="PSUM") as pp,
            tc.tile_pool(name="psum_soc", bufs=1, space="PSUM") as pps,
        ):
            # ---- constants / staged inputs ----
            ramp = cpool.tile([128, G * NC_CHUNK], MDT, tag="ramp")
            nc.gpsimd.iota(ramp[:, :], pattern=[[11, GS], [1, GS], [0, NC_CHUNK]],
                           base=12, channel_multiplier=0,
                           allow_small_or_imprecise_dtypes=True)
            xabs_sb = cpool.tile([128, 2 * MC], F32, tag="xabs")
            nc.sync.dma_start(xabs_sb[:, :], xabs_r[:, :])
            xm02 = cpool.tile([128, 2 * MC], F32, tag="xm02")
            nc.vector.tensor_scalar(xm02[:, :], xabs_sb[:, :], 0.2, None,
                                    op0=ALU.add)
            xnb_sb = cpool.tile([128, NC_CHUNK], F32, tag="xnb")
            nc.sync.dma_start(xnb_sb[:, :], xnb[:, :])
            ynb_sb = cpool.tile([128, NC_CHUNK], F32, tag="ynb")
            nc.sync.dma_start(ynb_sb[:, :], ynb[:, :])
            eye_sb = cpool.tile([128, N], MDT, tag="eye")
            nc.sync.dma_start(eye_sb[:, :], eye_r[:, :])
            h_sb = []
            for mc in range(MC):
                t = cpool.tile([128, RNN], MDT, tag=f"h{mc}")
                nc.sync.dma_start(t[:, :], h_all[mc * 128:(mc + 1) * 128, :])
                h_sb.append(t)
            wsoc_sb = cpool.tile([RNN, G * EMB], MDT, tag="wsoc")
            nc.sync.dma_start(wsoc_sb[:, :], wsoc_r[:, :])

            # ---- stage A: cell codes per neighbor chunk ----
            # code = 108 - t2x - 11*t2y, t2 = rint(relu(9 - relu(v + 0.5)))
            # valid bin (gx,gy) -> 12 + gx + 11*gy; anything else can't match.
            cell16 = []
            for mc in range(MC):
                vx = work.tile([128, NC_CHUNK], F32, tag="vx")
                nc.vector.tensor_scalar(vx[:, :], xnb_sb[:, :],
                                        xm02[:, 2 * mc:2 * mc + 1],
                                        -20.0,
                                        op0=ALU.subtract, op1=ALU.mult)
                vy = work.tile([128, NC_CHUNK], F32, tag="vy")
                nc.vector.tensor_scalar(vy[:, :], ynb_sb[:, :],
                                        xm02[:, 2 * mc + 1:2 * mc + 2],
                                        -20.0,
                                        op0=ALU.subtract, op1=ALU.mult)
                t1x = work.tile([128, NC_CHUNK], F32, tag="t1x")
                nc.scalar.activation(t1x[:, :], vx[:, :], ACT.Relu,
                                     bias=0.5, scale=1.0)
                t2x = work.tile([128, NC_CHUNK], F32, tag="t2x")
                nc.scalar.activation(t2x[:, :], t1x[:, :], ACT.Relu,
                                     bias=9.0, scale=-1.0)
                t1y = work.tile([128, NC_CHUNK], F32, tag="t1y")
                nc.scalar.activation(t1y[:, :], vy[:, :], ACT.Relu,
                                     bias=0.5, scale=1.0)
                t2y = work.tile([128, NC_CHUNK], F32, tag="t2y")
                nc.scalar.activation(t2y[:, :], t1y[:, :], ACT.Relu,
                                     bias=9.0, scale=-1.0)
                # f32->i32 converts round-to-nearest-even (HW verified)
                t2xi = work.tile([128, NC_CHUNK], I32, tag="t2xi")
                nc.vector.tensor_scalar(t2xi[:, :], t2x[:, :], 0.0, None,
                                        op0=ALU.add)
                t2yi = work.tile([128, NC_CHUNK], I32, tag="t2yi")
                nc.vector.tensor_scalar(t2yi[:, :], t2y[:, :], 0.0, None,
                                        op0=ALU.add)
                u = work.tile([128, NC_CHUNK], I32, tag="u")
                nc.vector.tensor_scalar(u[:, :], t2yi[:, :], -11, 108,
                                        op0=ALU.mult, op1=ALU.add)
                cc = cellpool.tile([128, NC_CHUNK], MDT, tag=f"cell{mc}")
                nc.vector.tensor_tensor(cc[:, :], u[:, :], t2xi[:, :],
                                        op=ALU.subtract)
                nc.vector.tensor_tensor(cc[:, :], cc[:, :],
                                        eye_sb[:, mc * 128:(mc + 1) * 128],
                                        op=ALU.add)
                cell16.append(cc)

            # ---- stage B: masks + social matmuls + pooling ----
            xin_ps = pp.tile([128, NC_CHUNK], F32, tag="xin_ps")
            for gg in range(N_GRP):
                soc_ps = pps.tile([128, G_GRP * NC_CHUNK], F32, tag="soc_ps")
                for mc in range(MC):
                    mask = maskpool.tile([128, G_GRP * NC_CHUNK], MDT, tag="mask")
                    cb = cell16[mc][:, :].unsqueeze(1).broadcast_to(
                        [128, G_GRP, NC_CHUNK])
                    nc.vector.tensor_tensor(
                        mask[:, :], cb,
                        ramp[:, gg * G_GRP * NC_CHUNK:(gg + 1) * G_GRP * NC_CHUNK],
                        op=ALU.is_equal)
                    for q in range(G_GRP * NC_CHUNK // 512):
                        nc.tensor.matmul(soc_ps[:, q * 512:(q + 1) * 512],
                                         h_sb[mc][:, :],
                                         mask[:, q * 512:(q + 1) * 512],
                                         start=(mc == 0), stop=(mc == MC - 1))
                soc_sb = socpool.tile([128, G_GRP * NC_CHUNK], MDT, tag="soc_sb")
                nc.scalar.activation(soc_sb[:, :], soc_ps[:, :], ACT.Copy,
                                     bias=0.0, scale=1.0)
                for gl in range(G_GRP):
                    g = gg * G_GRP + gl
                    nc.tensor.matmul(xin_ps[EMB:, :],
                                     wsoc_sb[:, g * EMB:(g + 1) * EMB],
                                     soc_sb[:, gl * NC_CHUNK:(gl + 1) * NC_CHUNK],
                                     start=(g == 0), stop=(g == G - 1))

            # ---- stage C: embedding ----
            wembT_sb = cpool.tile([2, EMB], F32, tag="wembT")
            nc.sync.dma_start(wembT_sb[:, :], wembT[:, :])
            xoffT_sb = cpool.tile([2, NC_CHUNK], F32, tag="xoffT")
            nc.sync.dma_start(xoffT_sb[:, :], xoffT[:, :])
            nc.tensor.matmul(xin_ps[:EMB, :], wembT_sb[:, :], xoffT_sb[:, :],
                             start=True, stop=True)
            b_es_sb = cpool.tile([128, 1], F32, tag="b_embsoc")
            nc.sync.dma_start(b_es_sb[:, :], b_embsoc[:, :])
            xinT = work.tile([128, NC_CHUNK], F32, tag="xinT")
            nc.scalar.activation(xinT[:, :], xin_ps[:, :], ACT.Relu,
                                 bias=b_es_sb[:, 0:1], scale=1.0)

            # ---- stage D: LSTM gates ----
            wihT_sb = cpool.tile([128, 4 * RNN], F32, tag="wihT")
            nc.sync.dma_start(wihT_sb[:, :], wihT[:, :])
            whhT_sb = cpool.tile([RNN, 4 * RNN], F32, tag="whhT")
            nc.sync.dma_start(whhT_sb[:, :], whhT[:, :])
            hT_sb = cpool.tile([RNN, NC_CHUNK], F32, tag="hT")
            nc.sync.dma_start(hT_sb[:, :], hT_c[:, :])
            cT_sb = cpool.tile([RNN, NC_CHUNK], F32, tag="cT")
            nc.sync.dma_start(cT_sb[:, :], cT_c[:, :])
            bgi_sb = cpool.tile([128, 4], F32, tag="bgates_ih")
            nc.sync.dma_start(bgi_sb[:, :], bgates_ih[:, :])
            bgh_sb = cpool.tile([128, 4], F32, tag="bgates_hh")
            nc.sync.dma_start(bgh_sb[:, :], bgates_hh[:, :])
            bg_sb = cpool.tile([128, 4], F32, tag="bgates")
            nc.vector.tensor_tensor(bg_sb[:, :], bgi_sb[:, :], bgh_sb[:, :],
                                    op=ALU.add)

            acts = []  # sigmoid(i), sigmoid(f), tanh(g), sigmoid(o)
            for q in range(4):
                g_ps = pp.tile([128, NC_CHUNK], F32, tag="g_ps")
                nc.tensor.matmul(g_ps[:, :], wihT_sb[:, q * RNN:(q + 1) * RNN],
                                 xinT[:, :], start=True, stop=False)
                nc.tensor.matmul(g_ps[:, :], whhT_sb[:, q * RNN:(q + 1) * RNN],
                                 hT_sb[:, :], start=False, stop=True)
                gq = work.tile([128, NC_CHUNK], F32, tag=f"gate{q}")
                func = ACT.Tanh if q == 2 else ACT.Sigmoid
                nc.scalar.activation(gq[:, :], g_ps[:, :], func,
                                     bias=bg_sb[:, q:q + 1], scale=1.0)
                acts.append(gq)

            fc = work.tile([128, NC_CHUNK], F32, tag="fc")
            nc.vector.tensor_tensor(fc[:, :], acts[1][:, :], cT_sb[:, :],
                                    op=ALU.mult)
            ig = work.tile([128, NC_CHUNK], F32, tag="ig")
            nc.vector.tensor_tensor(ig[:, :], acts[0][:, :], acts[2][:, :],
                                    op=ALU.mult)
            cnew = work.tile([128, NC_CHUNK], F32, tag="cnew")
            nc.vector.tensor_tensor(cnew[:, :], fc[:, :], ig[:, :], op=ALU.add)
            tc_t = work.tile([128, NC_CHUNK], F32, tag="tc")
            nc.scalar.activation(tc_t[:, :], cnew[:, :], ACT.Tanh,
                                 bias=0.0, scale=1.0)
            hn = work.tile([128, NC_CHUNK], F32, tag="hn")
            nc.vector.tensor_tensor(hn[:, :], acts[3][:, :], tc_t[:, :],
                                    op=ALU.mult)

            # ---- stage E: output projection ----
            woutT_sb = cpool.tile([RNN, 6 * NMIX], F32, tag="woutT")
            nc.sync.dma_start(woutT_sb[:, :], woutT[:, :])
            bout_sb = cpool.tile([6 * NMIX, 1], F32, tag="bout")
            nc.sync.dma_start(bout_sb[:, :], bout[:, :])
            out_ps = pp.tile([6 * NMIX, NC_CHUNK], F32, tag="out_ps")
            nc.tensor.matmul(out_ps[:, :], woutT_sb[:, :], hn[:, :],
                             start=True, stop=True)
            outT_sb = work.tile([6 * NMIX, NC_CHUNK], F32, tag="outT")
            nc.vector.tensor_scalar(outT_sb[:, :], out_ps[:, :],
                                    bout_sb[:, 0:1], None, op0=ALU.add)
            nc.sync.dma_start(outT[:, :], outT_sb[:, :])

    _split_multi_waits(nc)
    return nc


_NC_CACHE = None


def _get_program():
    global _NC_CACHE
    if _NC_CACHE is None:
        _NC_CACHE = _build_program()
    return _NC_CACHE


def _prep_inputs(xoff, xabs, h0, c0, W_emb, b_emb, W_soc, b_soc,
                 W_ih, W_hh, b_ih, b_hh, W_out, b_out):
    f32 = np.float32
    xoff = np.asarray(xoff, f32)
    xabs = np.asarray(xabs, f32)
    h = np.asarray(h0, f32)[0]
    c = np.asarray(c0, f32)[0]
    W_emb = np.asarray(W_emb, f32)
    W_soc = np.asarray(W_soc, f32)
    W_ih = np.asarray(W_ih, f32)
    W_hh = np.asarray(W_hh, f32)
    W_out = np.asarray(W_out, f32)

    xabs_r = np.ascontiguousarray(
        xabs.reshape(MC, 128, 2).transpose(1, 0, 2).reshape(128, 2 * MC))
    h_b = h.astype(MNP)
    wsoc_r = np.ascontiguousarray(
        W_soc.reshape(EMB, G, RNN).transpose(2, 1, 0).reshape(RNN, G * EMB)
    ).astype(MNP)
    wembT = np.ascontiguousarray(W_emb.T)
    b_embsoc = np.concatenate([np.asarray(b_emb, f32),
                               np.asarray(b_soc, f32)])[:, None]
    b_embsoc = np.ascontiguousarray(b_embsoc)
    wihT = np.ascontiguousarray(W_ih.T)
    whhT = np.ascontiguousarray(W_hh.T)
    bgates_ih = np.ascontiguousarray(np.asarray(b_ih, f32).reshape(4, RNN).T)
    bgates_hh = np.ascontiguousarray(np.asarray(b_hh, f32).reshape(4, RNN).T)
    woutT = np.ascontiguousarray(W_out.T)
    bout = np.ascontiguousarray(np.asarray(b_out, f32)[:, None])

    in_maps = []
    for k in range(NCORE):
        sl = slice(k * NC_CHUNK, (k + 1) * NC_CHUNK)
        eye_r = np.zeros((128, N), MNP)
        idx = np.arange(128)
        eye_r[idx, k * 128 + idx] = MNP(1000.0)
        in_maps.append({
            "xabs_r": xabs_r,
            "xnb": np.ascontiguousarray(
                np.broadcast_to(xabs[sl, 0][None, :], (128, NC_CHUNK))),
            "ynb": np.ascontiguousarray(
                np.broadcast_to(xabs[sl, 1][None, :], (128, NC_CHUNK))),
            "eye_r": eye_r,
            "h_all": h_b,
            "wsoc_r": wsoc_r,
            "wembT": wembT,
            "xoffT": np.ascontiguousarray(xoff[sl].T),
            "b_embsoc": b_embsoc,
            "wihT": wihT,
            "whhT": whhT,
            "bgates_ih": bgates_ih,
            "bgates_hh": bgates_hh,
            "hT_c": np.ascontiguousarray(h[sl].T),
            "cT_c": np.ascontiguousarray(c[sl].T),
            "woutT": woutT,
            "bout": bout,
        })
    return in_maps


def kernel(**inputs):
    nc = _get_program()
    in_maps = _prep_inputs(**inputs)
    res = run_bass_kernel_spmd(nc, in_maps, list(range(NCORE)))
    outT = np.concatenate([res.results[k]["outT"] for k in range(NCORE)],
                          axis=1)            # [120, 1024]
    out = np.ascontiguousarray(outT.T)       # [1024, 120]
    return tuple(np.ascontiguousarray(out[:, i * NMIX:(i + 1) * NMIX])
                 for i in range(6))
